# revision 28
# baseline (speedup 1.0000x reference)
"""Trainium2 Bass kernel for AdaptiveMixtureOfExperts (top-2 SwiGLU MoE).

Strategy (expert-parallel with FF-split load balancing):
  - Host computes the tiny router (x @ Wr, top-2, softmax) with jax-on-CPU ops
    that bit-match the reference, then shards tokens by routed expert.
  - Each expert's FFN is split in half along D_FF; each NeuronCore runs two
    half-FFN "sections": section A = FF-half h of one of the 4 largest
    experts, section B = FF-half h of one of the 4 smallest (cores 0-3 take
    h=0, cores 4-7 take h=1).  This balances per-core work to the average of
    a large+small expert instead of the max expert.
        hT = W1h.T @ xgT            (ff on partitions, tokens on free dim)
        uT = (a + b1a) * silu(g + b1g)
        yT_partial = W2h.T @ uT     (+ b2 on half-0 cores only)
  - Host sums the two half contributions per expert, applies the top-2
    combine weights, and scatter-adds into the full [B, S, D] output.

Shapes hardcoded for the problem instance:
  x:[2,2048,1024] f32, Wr:[1024,8], temp:[1], W1:[8,1024,4096], b1:[8,4096],
  W2:[8,2048,1024], b2:[8,1024].  TOP_K=2, 8 experts on 8 cores.
"""

import os

import numpy as np
import ml_dtypes

D_MODEL = 1024
D_FF = 2048
NUM_EXPERTS = 8
TOP_K = 2
P = 128          # partitions
NT = 512         # token tile (moving free dim per matmul)
N_CORES = 8
FH = D_FF // 2   # ff half

_NC_CACHE = {}
LAST_RESULTS = None  # test harness introspection


def _build_nc(CA: int, CB: int, use_silu: bool = True):
    """Per-core Bass graph: two half-FF FFN sections of CA and CB tokens.

    use_silu=False decomposes silu into sigmoid+mul (CoreSim has no Silu).
    """
    import concourse.mybir as mybir
    import concourse.tile as tile
    from concourse import bacc
    from concourse.bass import ts

    f32 = mybir.dt.float32
    bf16 = mybir.dt.bfloat16
    AF = mybir.ActivationFunctionType

    K1 = D_MODEL // P          # 8 k-tiles for matmul1
    K2 = FH // P               # 8 k-tiles for matmul2 (half ff)
    NF1 = 2 * FH // P          # 16 ff tiles of hT per section (a+g)
    NO = D_MODEL // P          # 8 out tiles of yT
    WCHUNK = 512

    nc = bacc.Bacc()
    xT = {}
    w1 = {}
    w2 = {}
    b1t = {}
    outp = {}
    secs = [("a", CA), ("b", CB)]
    for s, C in secs:
        xT[s] = nc.declare_dram_parameter(f"x{s}T", [D_MODEL, C], bf16, isOutput=False)
        w1[s] = nc.declare_dram_parameter(f"w1{s}", [D_MODEL, 2 * FH], bf16, isOutput=False)
        w2[s] = nc.declare_dram_parameter(f"w2{s}", [FH, D_MODEL], bf16, isOutput=False)
        b1t[s] = nc.declare_dram_parameter(f"b1t{s}", [P, NF1], f32, isOutput=False)
        # partial y without b2 (host adds the bias once per expert), bf16 to
        # halve output DMA bytes
        outp[s] = nc.declare_dram_parameter(f"out{s}", [D_MODEL, C], bf16, isOutput=True)

    with tile.TileContext(nc) as tc:
        with (
            tc.tile_pool(name="weights", bufs=1) as wpool,
            tc.tile_pool(name="acts", bufs=2) as upool,
            tc.tile_pool(name="epilogue", bufs=4) as epool,
            tc.tile_pool(name="psA", bufs=2, space="PSUM") as psa_pool,
            tc.tile_pool(name="psG", bufs=2, space="PSUM") as psg_pool,
            tc.tile_pool(name="psY", bufs=2, space="PSUM") as psy_pool,
        ):
            # ---- small early inputs on qACT (biases) ----
            b1_sb = {}
            for s, C in secs:
                b1_sb[s] = wpool.tile([P, NF1], f32, name=f"b1_sb{s}", tag=f"b1{s}")
                nc.scalar.dma_start(out=b1_sb[s][:], in_=b1t[s][:])

            xg_sb = {}
            w1_sb = {}
            w2_sb = {}
            for s, C in secs:
                xg_sb[s] = [
                    wpool.tile([P, C], bf16, name=f"xg_sb{s}{k}", tag=f"xg{s}{k}")
                    for k in range(K1)
                ]
                w1_sb[s] = [
                    wpool.tile([P, 2 * FH], bf16, name=f"w1_sb{s}{k}", tag=f"w1{s}{k}")
                    for k in range(K1)
                ]
                w2_sb[s] = [
                    wpool.tile([P, D_MODEL], bf16, name=f"w2_sb{s}{k}", tag=f"w2{s}{k}")
                    for k in range(K2)
                ]

            # PE warmup: dummy matmuls on a zeroed tile keep the PE busy (and
            # open the HAM clock gate to 2.4 GHz) until the first input DMAs
            # land (~8us fixed preamble + first chunks).
            warm = wpool.tile([P, NT], bf16, name="warm")
            nc.gpsimd.memset(warm[:], 0.0)
            ps_w = psa_pool.tile([P, NT], f32, name="ps_warm", tag="psa")
            for _ in range(52):
                nc.tensor.matmul(ps_w[:], warm[:, :P], warm[:], start=True, stop=True)

            # ---- bulk inputs on qSP in exact PE consumption order.
            # (qACT is unusable for inputs: dma_start issue on the ACT engine
            # stream blocks the PSUM-drain epilogue ACTs behind it.)
            def eng(k, K):
                return nc.sync

            def emit_input_dmas(s, C):
                # all token tiles first (needed upfront by the i-outer loop),
                # then w1 chunks in column order, then w2
                for t in range((C + NT - 1) // NT):
                    off = t * NT
                    Nt = min(NT, C - off)
                    for k in range(K1):
                        nc.sync.dma_start(
                            out=xg_sb[s][k][:, off:off + Nt],
                            in_=xT[s][k * P:(k + 1) * P, off:off + Nt],
                        )
                for c0 in range(0, 2 * FH, WCHUNK):
                    for k in range(K1):
                        nc.sync.dma_start(
                            out=w1_sb[s][k][:, c0:c0 + WCHUNK],
                            in_=w1[s][k * P:(k + 1) * P, c0:c0 + WCHUNK],
                        )
                for k in range(K2):
                    for c0 in range(0, D_MODEL, WCHUNK):
                        nc.sync.dma_start(
                            out=w2_sb[s][k][:, c0:c0 + WCHUNK],
                            in_=w2[s][k * P:(k + 1) * P, c0:c0 + WCHUNK],
                        )

            for s, C in secs:
                emit_input_dmas(s, C)

            # ---- main loops ----
            # W1 columns host-permuted to [a_0 | g_0 | a_1 | g_1 | ...] so the
            # PE reads w1_sb left-to-right.  matmul1 iterates i-outer/t-inner:
            # all token tiles consume one weight block before moving on, so
            # the weight-DMA demand rate is ~halved and tokens (cheap, small)
            # are needed upfront instead of mid-stream.
            uT = {}

            def emit_mm1(s, C):
                n_t = (C + NT - 1) // NT
                for t in range(n_t):
                    uT[(s, t)] = upool.tile(
                        [P, K2, NT], bf16, name=f"uT{s}{t}", tag="uT", bufs=4)
                for i in range(K2):
                    for t in range(n_t):
                        off = t * NT
                        Nt = min(NT, C - off)
                        ps_a = psa_pool.tile(
                            [P, NT], f32, name=f"psa{s}{t}_{i}", tag="psa")
                        for k in range(K1):
                            nc.tensor.matmul(
                                ps_a[:, :Nt],
                                w1_sb[s][k][:, ts(2 * i, P)],
                                xg_sb[s][k][:, off:off + Nt],
                                start=(k == 0),
                                stop=(k == K1 - 1),
                            )
                        ps_g = psg_pool.tile(
                            [P, NT], f32, name=f"psg{s}{t}_{i}", tag="psg")
                        for k in range(K1):
                            nc.tensor.matmul(
                                ps_g[:, :Nt],
                                w1_sb[s][k][:, ts(2 * i + 1, P)],
                                xg_sb[s][k][:, off:off + Nt],
                                start=(k == 0),
                                stop=(k == K1 - 1),
                            )
                        a_t = epool.tile([P, NT], bf16, name=f"a{s}{t}_{i}", tag="a")
                        nc.scalar.activation(
                            a_t[:, :Nt], ps_a[:, :Nt], AF.Identity,
                            bias=b1_sb[s][:, 2 * i:2 * i + 1],
                        )
                        g_t = epool.tile([P, NT], bf16, name=f"g{s}{t}_{i}", tag="g")
                        if use_silu:
                            nc.scalar.activation(
                                g_t[:, :Nt], ps_g[:, :Nt], AF.Silu,
                                bias=b1_sb[s][:, 2 * i + 1:2 * i + 2],
                            )
                        else:
                            s_t = epool.tile(
                                [P, NT], bf16, name=f"s{s}{t}_{i}", tag="s")
                            nc.scalar.activation(
                                s_t[:, :Nt], ps_g[:, :Nt], AF.Sigmoid,
                                bias=b1_sb[s][:, 2 * i + 1:2 * i + 2],
                            )
                            gb_t = epool.tile(
                                [P, NT], bf16, name=f"gb{s}{t}_{i}", tag="gb")
                            nc.scalar.activation(
                                gb_t[:, :Nt], ps_g[:, :Nt], AF.Identity,
                                bias=b1_sb[s][:, 2 * i + 1:2 * i + 2],
                            )
                            nc.vector.tensor_mul(
                                g_t[:, :Nt], gb_t[:, :Nt], s_t[:, :Nt])
                        nc.vector.tensor_mul(
                            uT[(s, t)][:, i, :Nt], a_t[:, :Nt], g_t[:, :Nt])

            def emit_mm2(s, C):
                for t in range((C + NT - 1) // NT):
                    off = t * NT
                    Nt = min(NT, C - off)
                    for m in range(NO):
                        ps_y = psy_pool.tile(
                            [P, NT], f32, name=f"psy{s}{t}_{m}", tag="psy")
                        for k in range(K2):
                            nc.tensor.matmul(
                                ps_y[:, :Nt],
                                w2_sb[s][k][:, ts(m, P)],
                                uT[(s, t)][:, k, :Nt],
                                start=(k == 0),
                                stop=(k == K2 - 1),
                            )
                        # psum drain on DVE (idle), output via qSP behind the
                        # inputs: keeps the ScalarE stream free for the a/g
                        # drains (dma_start issue on ACT blocks them) and
                        # avoids SWDGE SBUF-read contention with the PE.
                        y_t = epool.tile([P, NT], bf16, name=f"y{s}{t}_{m}",
                                         tag="y", bufs=12)
                        nc.vector.tensor_copy(y_t[:, :Nt], ps_y[:, :Nt])
                        nc.sync.dma_start(
                            out=outp[s][m * P:(m + 1) * P, off:off + Nt],
                            in_=y_t[:, :Nt],
                        )

            emit_mm1("a", CA)
            emit_mm2("a", CA)
            emit_mm1("b", CB)
            emit_mm2("b", CB)

    nc.compile()
    return nc


def _route_tokens(xf, Wr, temp):
    """Bit-match the reference's router on CPU jax: logits, top-2, softmax."""
    import jax
    import jax.numpy as jnp

    cpu = jax.devices("cpu")[0]
    with jax.default_device(cpu):
        xj = jnp.asarray(xf)
        logits = (xj @ jnp.asarray(Wr)) / jnp.asarray(temp)
        topw, topi = jax.lax.top_k(logits, TOP_K)
        topw = jax.nn.softmax(topw, axis=-1)
    return np.asarray(topi), np.asarray(topw)


def _pad32(n):
    return max(P, ((n + 31) // 32) * 32)


def kernel(**inputs) -> np.ndarray:
    global LAST_RESULTS
    from concourse.bass_utils import run_bass_kernel_spmd

    x = np.asarray(inputs["x"], dtype=np.float32)
    Wr = np.asarray(inputs["Wr"], dtype=np.float32)
    temp = np.asarray(inputs["temp"], dtype=np.float32)
    W1 = np.asarray(inputs["W1"], dtype=np.float32)
    b1 = np.asarray(inputs["b1"], dtype=np.float32)
    W2 = np.asarray(inputs["W2"], dtype=np.float32)
    b2 = np.asarray(inputs["b2"], dtype=np.float32)

    B, S, D = x.shape
    T = B * S
    xf = x.reshape(T, D)

    topi, topw = _route_tokens(xf, Wr, temp)

    # Per-expert token lists and combine weights.
    tok_idx = []
    tok_w = []
    for e in range(NUM_EXPERTS):
        mask = topi == e                       # [T, K]
        sel = mask.any(axis=1)
        idx = np.nonzero(sel)[0]
        w = (topw * mask).sum(axis=1)[idx]
        tok_idx.append(idx)
        tok_w.append(w.astype(np.float32))

    counts = np.array([len(i) for i in tok_idx])
    order = np.argsort(-counts, kind="stable")
    bigs = list(order[:4])                     # section A experts
    smalls = list(order[4:])                   # section B experts
    CA = _pad32(max(counts[e] for e in bigs))
    CB = _pad32(max(counts[e] for e in smalls))

    # a/g interleave within a ff half: [a_0 | g_0 | a_1 | g_1 | ...]
    def w1_cols(h):
        cols = []
        for j in range(h * (FH // P), (h + 1) * (FH // P)):
            cols.append(np.arange(j * P, (j + 1) * P))            # a_j
            cols.append(np.arange(D_FF + j * P, D_FF + (j + 1) * P))  # g_j
        return np.concatenate(cols)

    cols_h = [w1_cols(0), w1_cols(1)]

    bf16 = ml_dtypes.bfloat16

    def xgT_of(e, C):
        idx = tok_idx[e]
        xg = np.zeros((C, D), dtype=np.float32)
        xg[: len(idx)] = xf[idx]
        return np.ascontiguousarray(xg.T).astype(bf16)

    xgT_cache = {e: xgT_of(e, CA if e in bigs else CB) for e in range(NUM_EXPERTS)}

    in_maps = []
    for c in range(N_CORES):
        h = c // 4
        m = {}
        for s, elist in (("a", bigs), ("b", smalls)):
            e = elist[c % 4]
            cols = cols_h[h]
            m[f"x{s}T"] = xgT_cache[e]
            m[f"w1{s}"] = np.ascontiguousarray(W1[e][:, cols]).astype(bf16)
            m[f"w2{s}"] = np.ascontiguousarray(
                W2[e][h * FH:(h + 1) * FH, :]).astype(bf16)
            m[f"b1t{s}"] = np.ascontiguousarray(
                b1[e][cols].reshape(2 * FH // P, P).T)
        in_maps.append(m)

    key = (CA, CB)
    if key not in _NC_CACHE:
        _NC_CACHE[key] = _build_nc(CA, CB)
    nc = _NC_CACHE[key]

    trace = bool(os.environ.get("MOE_KERNEL_TRACE"))
    kwargs = {}
    if trace:
        kwargs = dict(trace=True, trace_cores=list(range(N_CORES)))
    res = run_bass_kernel_spmd(nc, in_maps, core_ids=list(range(N_CORES)), **kwargs)
    LAST_RESULTS = res

    out = np.zeros((T, D), dtype=np.float32)
    for s, elist in (("a", bigs), ("b", smalls)):
        for i, e in enumerate(elist):
            idx = tok_idx[e]
            if len(idx) == 0:
                continue
            y0 = np.asarray(res.results[i]["out" + s]).astype(np.float32)
            y1 = np.asarray(res.results[i + 4]["out" + s]).astype(np.float32)
            y = (y0 + y1)[:, : len(idx)].T + b2[e]
            out[idx] += y * tok_w[e][:, None]

    return out.reshape(B, S, D)
